# revision 17
# baseline (speedup 1.0000x reference)
"""GCN (2-layer, PyG GCNConv semantics) on 8 Trainium2 NeuronCores.

Strategy (sharding_hint: shard nodes across cores, partition edges by dst):
  - Nodes sharded contiguously: core c owns dst rows [c*NP, (c+1)*NP).
  - Layer matmuls computed on the owning core (fp16 operands, fp32 PSUM).
  - Hidden tables (h1' = dis*x@W1, h2' = dis*relu(z1)@W2) are AllGathered
    in 4 pieces (overlapped with compute) so every core can gather
    messages for its own edges locally.
  - The symmetric norm dis[s]*dis[d] is factored: table rows are
    pre-scaled by dis[v]; the window PSUM is scaled by dis[d] on the way
    out (ACT scale); the bias is injected as (b/dis[d]) via a K=1 matmul
    that also initializes the accumulation group.
  - Aggregation out[dst] += h'[src_e] runs per 128-dst window: dma_gather
    pulls h'[src] rows for the window's non-self edges into SBUF (128
    edges per chunk), a 0/1 selection mask G[e, d] = (dst_rel[e]==d) is
    built with one DVE tensor_tensor per (window, half), and TensorE
    accumulates G.T @ msg into the window's PSUM tile.  Self-loops are
    the own-shard diagonal: one identity matmul on a contiguous DMA of
    the own h' tile.
  - All cores run one identical program: each (window, src-half) edge
    group is padded to CH_wh[w,h] chunks of 128 tokens, CH_wh being the
    max chunk count over the 8 cores (pad tokens have dst_rel=255 so the
    mask kills them).
"""

import math

import numpy as np

M = 8  # cores
P = 128  # partitions
AGP = 4  # all-gather pieces
BW = 4  # windows per gather batch


def _prep(x, W1, b1, W2, b2, edge_index):
    """Host-side sharding/layout (index manipulation + dtype casts only)."""
    N, IN = x.shape
    HID = W1.shape[1]
    OUT = W2.shape[1]
    OUTP = P
    assert N % M == 0
    NP = N // M
    NPAD = math.ceil(NP / P) * P
    NT = NPAD // P
    VROWS = M * NPAD
    HALF = VROWS // 2
    assert HALF <= 32768, "gather idx must fit int16"

    src = np.asarray(edge_index[0], dtype=np.int64)
    dst = np.asarray(edge_index[1], dtype=np.int64)
    deg = (np.bincount(dst, minlength=N) + 1).astype(np.float32)
    dis = 1.0 / np.sqrt(deg)

    batches = [list(range(s, min(s + BW, NT))) for s in range(0, NT, BW)]
    nb = len(batches)
    pieces = []
    per = math.ceil(nb / AGP)
    for s in range(0, nb, per):
        pieces.append((s, min(s + per, nb)))
    piece_wins = [sum(len(batches[b]) for b in range(lo, hi)) for lo, hi in pieces]
    piece_rows = [wn * P for wn in piece_wins]
    piece_win_start = np.cumsum([0] + piece_wins)
    piece_base = np.cumsum([0] + [r * M for r in piece_rows])

    win_of_l = np.arange(NPAD) // P
    piece_of_win = np.zeros(NT, dtype=np.int64)
    for j in range(len(pieces)):
        piece_of_win[piece_win_start[j] : piece_win_start[j + 1]] = j

    sc, sl = src // NP, src % NP
    pj = piece_of_win[win_of_l[sl]]
    g = (
        piece_base[pj]
        + sc * np.array(piece_rows)[pj]
        + (sl - piece_win_start[pj] * P)
    )
    half = g // HALF
    lidx = g - half * HALF

    owner = dst // NP
    ldst = dst - owner * NP
    win = ldst // P

    gid = (owner * NT + win) * 2 + half
    order = np.lexsort((lidx, gid))
    lidx_s = lidx[order]
    drel_s = (ldst[order] % P).astype(np.float16)

    ngroups = M * NT * 2
    counts = np.bincount(gid[order], minlength=ngroups).reshape(M, NT, 2)
    # per-(window, half) chunk count: max over cores (identical SPMD program)
    CH_wh = np.ceil(counts.max(axis=0) / P).astype(np.int64)  # [NT, 2]
    CHMAX = max(1, int(CH_wh.max()))

    # token slots / chunk columns in kernel iteration order:
    # batch -> half -> window-in-batch -> chunk
    slot_base = np.zeros((NT, 2), dtype=np.int64)
    col_of = np.zeros((NT, 2), dtype=np.int64)
    tb = 0
    cb = 0
    for bwins in batches:
        for h in (0, 1):
            for w in bwins:
                slot_base[w, h] = tb
                col_of[w, h] = cb
                tb += int(CH_wh[w, h]) * P
                cb += int(CH_wh[w, h])
    TOK = int(tb)
    NCHUNK = int(cb)

    group_starts = np.zeros(ngroups + 1, dtype=np.int64)
    np.cumsum(counts.reshape(-1), out=group_starts[1:])

    in_maps = []
    f16 = np.float16
    w1f = np.ascontiguousarray(W1, dtype=f16)
    b1f = np.ascontiguousarray(
        np.broadcast_to(np.asarray(b1, dtype=f16).reshape(1, HID), (P, HID))
    )
    w2f = np.zeros((HID, OUTP), dtype=f16)
    w2f[:, :OUT] = W2.astype(f16)
    b2f = np.zeros((P, OUTP), dtype=f16)
    b2f[:, :OUT] = np.asarray(b2, dtype=f16).reshape(1, OUT)
    iota_np = np.ascontiguousarray(
        np.broadcast_to(
            np.tile(np.arange(P, dtype=f16), CHMAX).reshape(1, CHMAX * P),
            (P, CHMAX * P),
        )
    )

    for c in range(M):
        xt = np.zeros((IN, NPAD), dtype=f16)
        xt[:, :NP] = x[c * NP : (c + 1) * NP].T
        idx16 = np.zeros(TOK, dtype=np.int16)
        drel = np.full(TOK, 255.0, dtype=np.float16)
        for w in range(NT):
            for h in (0, 1):
                gi = (c * NT + w) * 2 + h
                s0, s1 = group_starts[gi], group_starts[gi + 1]
                k = s1 - s0
                base = slot_base[w, h]
                idx16[base : base + k] = lidx_s[s0:s1]
                drel[base : base + k] = drel_s[s0:s1]
        idx_w = np.tile(idx16.reshape(-1, 16).T, (8, 1))
        grel = np.ascontiguousarray(drel.reshape(-1, P).T)
        dloc = np.ones(NPAD, np.float32)
        dloc[:NP] = dis[c * NP : (c + 1) * NP]
        disS = np.ascontiguousarray(dloc.reshape(NT, P).T)
        invdC = np.ascontiguousarray((1.0 / dloc).astype(np.float32).reshape(NT, P).T)
        invd = (1.0 / dloc).astype(f16).reshape(1, NPAD)
        in_maps.append(
            {
                "xt": xt,
                "w1": w1f,
                "b1": b1f,
                "w2": w2f,
                "b2": b2f,
                "iota": np.array(iota_np),
                "idx": np.ascontiguousarray(idx_w),
                "grel": grel,
                "disS": disS,
                "invdC": invdC,
                "invd": invd,
            }
        )

    meta = dict(
        N=N, IN=IN, HID=HID, OUT=OUT, OUTP=OUTP, NP=NP, NPAD=NPAD, NT=NT,
        VROWS=VROWS, HALF=HALF, CHMAX=CHMAX, TOK=TOK, NCHUNK=NCHUNK,
        CH_wh=[[int(CH_wh[w, 0]), int(CH_wh[w, 1])] for w in range(NT)],
        col_of=[[int(col_of[w, 0]), int(col_of[w, 1])] for w in range(NT)],
        batches=batches, piece_rows=piece_rows,
        piece_win_start=[int(v) for v in piece_win_start],
        piece_base=[int(v) for v in piece_base],
    )
    return in_maps, meta


def _build(meta):
    import os

    import concourse.mybir as mybir
    import concourse.tile as tile
    from concourse import bacc
    from concourse.bass import ts
    from concourse.masks import make_identity

    IN, HID, OUT, OUTP = meta["IN"], meta["HID"], meta["OUT"], meta["OUTP"]
    NPAD, NT, VROWS, HALF = meta["NPAD"], meta["NT"], meta["VROWS"], meta["HALF"]
    CHMAX, TOK, NCHUNK = meta["CHMAX"], meta["TOK"], meta["NCHUNK"]
    CH_wh = meta["CH_wh"]
    col_of = meta["col_of"]
    batches = meta["batches"]
    piece_rows = meta["piece_rows"]
    piece_win_start = meta["piece_win_start"]
    piece_base = meta["piece_base"]
    NPIECE = len(piece_rows)
    KT = IN // P
    HC = HID // P
    f16 = mybir.dt.float16
    f32 = mybir.dt.float32

    NQ = 4
    GN = 1024
    SP = os.environ.get("GCN_SP", "1") == "1"
    nc = bacc.Bacc(
        "TRN2",
        target_bir_lowering=False,
        debug=False,
        num_devices=M,
        num_swdge_queues=NQ,
    )

    xt_d = nc.dram_tensor("xt", [IN, NPAD], f16, kind="ExternalInput")
    w1_d = nc.dram_tensor("w1", [IN, HID], f16, kind="ExternalInput")
    b1_d = nc.dram_tensor("b1", [P, HID], f16, kind="ExternalInput")
    w2_d = nc.dram_tensor("w2", [HID, OUTP], f16, kind="ExternalInput")
    b2_d = nc.dram_tensor("b2", [P, OUTP], f16, kind="ExternalInput")
    iota_d = nc.dram_tensor("iota", [P, CHMAX * P], f16, kind="ExternalInput")
    idx_d = nc.dram_tensor("idx", [P, TOK // 16], mybir.dt.int16, kind="ExternalInput")
    grel_d = nc.dram_tensor("grel", [P, NCHUNK], f16, kind="ExternalInput")
    disS_d = nc.dram_tensor("disS", [P, NT], f32, kind="ExternalInput")
    invd_d = nc.dram_tensor("invd", [1, NPAD], f16, kind="ExternalInput")
    invdC_d = nc.dram_tensor("invdC", [P, NT], f32, kind="ExternalInput")
    out_d = nc.dram_tensor("out", [NPAD, OUT], f32, kind="ExternalOutput")

    h1_loc = [
        nc.dram_tensor(f"h1_loc{j}", [piece_rows[j], HID], f16)
        for j in range(NPIECE)
    ]
    h2_loc = [
        nc.dram_tensor(f"h2_loc{j}", [piece_rows[j], OUTP], f16)
        for j in range(NPIECE)
    ]
    h1_gl = nc.dram_tensor("h1_gl", [VROWS, HID], f16, addr_space="Shared")
    h2_gl = nc.dram_tensor("h2_gl", [VROWS, OUTP], f16, addr_space="Shared")

    rg = [list(range(M))]

    def win_piece(w):
        for j in range(NPIECE):
            if piece_win_start[j] <= w < piece_win_start[j + 1]:
                return j, w - piece_win_start[j]
        raise AssertionError(w)

    with tile.TileContext(nc) as tc:
        with (
            tc.tile_pool(name="const", bufs=1) as cp,
            tc.tile_pool(name="work", bufs=3) as wp,
            tc.tile_pool(name="gpool", bufs=4) as gp,
            tc.tile_pool(name="idxp", bufs=2) as idxp,
            tc.tile_pool(name="psum", bufs=2, space="PSUM") as pp,
        ):
            # ---- constants ----
            w1t = cp.tile([P, KT, HID], f16)
            nc.sync.dma_start(
                out=w1t[:], in_=w1_d[:, :].rearrange("(k p) h -> p k h", p=P)
            )
            w2t = cp.tile([P, HC, OUTP], f16)
            nc.sync.dma_start(
                out=w2t[:], in_=w2_d[:, :].rearrange("(k p) o -> p k o", p=P)
            )
            iota_t = cp.tile([P, CHMAX * P], f16)
            nc.sync.dma_start(out=iota_t[:], in_=iota_d[:, :])
            ident = cp.tile([P, P], f16)
            make_identity(nc, ident[:])
            b1s = cp.tile([P, HID], f16)
            nc.sync.dma_start(out=b1s[:], in_=b1_d[:, :])
            b2s = cp.tile([P, OUTP], f16)
            nc.sync.dma_start(out=b2s[:], in_=b2_d[:, :])
            grelS = cp.tile([P, NCHUNK], f16)
            nc.sync.dma_start(out=grelS[:], in_=grel_d[:, :])
            disS = cp.tile([P, NT], f32)
            nc.sync.dma_start(out=disS[:], in_=disS_d[:, :])
            invd = cp.tile([1, NPAD], f16)
            nc.sync.dma_start(out=invd[:], in_=invd_d[:, :])
            invdC = cp.tile([P, NT], f32)
            nc.sync.dma_start(out=invdC[:], in_=invdC_d[:, :])

            # ---- stage 1 ----
            for nt in range(NT):
                j, wo = win_piece(nt)
                xtt = wp.tile([P, KT, P], f16, tag="xtt")
                nc.sync.dma_start(
                    out=xtt[:],
                    in_=xt_d[:, ts(nt, P)].rearrange("(k p) n -> p k n", p=P),
                )
                ph = pp.tile([P, HID], f32, tag="acc256")
                for k in range(KT):
                    nc.tensor.matmul(
                        ph[:],
                        lhsT=xtt[:, k, :],
                        rhs=w1t[:, k, :],
                        start=(k == 0),
                        stop=(k == KT - 1),
                    )
                h1s = wp.tile([P, HID], f16, tag="h1s")
                nc.scalar.activation(
                    h1s[:], ph[:], mybir.ActivationFunctionType.Copy,
                    scale=disS[:, nt : nt + 1],
                )
                nc.sync.dma_start(out=h1_loc[j][ts(wo, P), :], in_=h1s[:])

            # ---- stage 2: chunked AllGather h1 ----
            for j in range(NPIECE):
                nc.gpsimd.collective_compute(
                    "AllGather",
                    mybir.AluOpType.bypass,
                    replica_groups=rg,
                    ins=[h1_loc[j].ap().opt()],
                    outs=[h1_gl[piece_base[j] : piece_base[j + 1], :].opt()],
                )

            qctr = [0]

            def build_GW(w, h):
                chw = CH_wh[w][h]
                c0 = col_of[w][h]
                GW = gp.tile([P, chw * P], f16, tag="GW")
                nc.vector.tensor_tensor(
                    out=GW[:].rearrange("p (c e) -> p c e", e=P),
                    in0=iota_t[:, : chw * P].rearrange("p (c e) -> p c e", e=P),
                    in1=grelS[:, c0 : c0 + chw].to_broadcast([P, chw, P]),
                    op=mybir.AluOpType.is_equal,
                )
                return GW

            def window_accum(w, msgs, moff, acc, bvec, own_loc, own_elem):
                """(self + bias/dis) identity matmul + edge-chunk matmuls."""
                j, wo = win_piece(w)
                own = wp.tile([P, own_elem], f16, tag=f"own{own_elem}")
                nc.sync.dma_start(out=own[:], in_=own_loc[j][ts(wo, P), :])
                ownb = wp.tile([P, own_elem], f16, tag=f"ownb{own_elem}")
                # ownb = bvec * (1/dis[d]) + own   (bias pre-divided by dis)
                nc.vector.scalar_tensor_tensor(
                    out=ownb[:],
                    in0=bvec[:],
                    scalar=invdC[:, w : w + 1],
                    in1=own[:],
                    op0=mybir.AluOpType.mult,
                    op1=mybir.AluOpType.add,
                )
                mms = []
                for h in (0, 1):
                    if CH_wh[w][h] == 0:
                        continue
                    GW = build_GW(w, h)
                    for c in range(CH_wh[w][h]):
                        mms.append((GW, h, c))
                nc.tensor.matmul(
                    acc[:], lhsT=ident[:], rhs=ownb[:],
                    start=True, stop=(len(mms) == 0),
                )
                for k, (GW, h, c) in enumerate(mms):
                    nc.tensor.matmul(
                        acc[:],
                        lhsT=GW[:, ts(c, P)],
                        rhs=msgs[h][:, moff[h] + c, :],
                        start=False,
                        stop=(k == len(mms) - 1),
                    )

            def agg_stage(table, elem, msg_pool, msg_tag, consume, post_window):
                tok_base = 0
                for bwins in batches:
                    chA = sum(CH_wh[w][0] for w in bwins)
                    chB = sum(CH_wh[w][1] for w in bwins)
                    btokA, btokB = chA * P, chB * P
                    btot = btokA + btokB
                    idx_t = idxp.tile([P, btot // 16], mybir.dt.int16, tag="idx")
                    nc.sync.dma_start(
                        out=idx_t[:],
                        in_=idx_d[:, tok_base // 16 : (tok_base + btot) // 16],
                    )
                    msgs = []
                    for h, btok in ((0, btokA), (1, btokB)):
                        nch = btok // P
                        if nch == 0:
                            msgs.append(None)
                            continue
                        mt = msg_pool.tile([P, nch, elem], f16, tag=msg_tag)
                        lo = h * HALF
                        i00 = 0 if h == 0 else btokA
                        for off in range(0, btok, GN):
                            gn = min(GN, btok - off)
                            i0 = i00 + off
                            nc.gpsimd.dma_gather(
                                out_ap=mt[:, off // P : (off + gn) // P, :],
                                in_ap=table[lo : lo + HALF, :],
                                idxs_ap=idx_t[:, i0 // 16 : (i0 + gn) // 16],
                                num_idxs=gn,
                                num_idxs_reg=gn,
                                elem_size=elem,
                                queue_num=qctr[0] % NQ,
                                single_packet=SP,
                            )
                            qctr[0] += 1
                        msgs.append(mt)
                    coff = [0, 0]
                    for w in bwins:
                        consume(w, msgs, (coff[0], coff[1]))
                        coff[0] += CH_wh[w][0]
                        coff[1] += CH_wh[w][1]
                        if post_window is not None:
                            post_window(w)
                    tok_base += btot

            # ---- stage 3 + chunked AllGather h2 ----
            def stage3_window(w, msgs, moff):
                j, wo = win_piece(w)
                pz = pp.tile([P, HID], f32, tag="acc256")
                window_accum(w, msgs, moff, pz, b1s, h1_loc, HID)
                z1r = wp.tile([P, HID], f16, tag="z1r")
                nc.scalar.activation(
                    z1r[:], pz[:], mybir.ActivationFunctionType.Relu,
                    scale=disS[:, w : w + 1],
                )
                ph2 = pp.tile([P, OUTP], f32, tag="acc128b")
                for k in range(HC):
                    pt = pp.tile([P, P], f16, tag="acc128t")
                    nc.tensor.transpose(pt[:], z1r[:, ts(k, P)], ident[:])
                    zt = wp.tile([P, P], f16, tag="zt")
                    nc.vector.tensor_copy(zt[:], pt[:])
                    nc.tensor.matmul(
                        ph2[:],
                        lhsT=zt[:],
                        rhs=w2t[:, k, :],
                        start=(k == 0),
                        stop=(k == HC - 1),
                    )
                h2s = wp.tile([P, OUTP], f16, tag="h2s")
                nc.scalar.activation(
                    h2s[:], ph2[:], mybir.ActivationFunctionType.Copy,
                    scale=disS[:, w : w + 1],
                )
                nc.sync.dma_start(out=h2_loc[j][ts(wo, P), :], in_=h2s[:])

            done_pieces = set()

            def fire_ag2(w):
                j, wo = win_piece(w)
                if wo == piece_rows[j] // P - 1 and j not in done_pieces:
                    done_pieces.add(j)
                    nc.gpsimd.collective_compute(
                        "AllGather",
                        mybir.AluOpType.bypass,
                        replica_groups=rg,
                        ins=[h2_loc[j].ap().opt()],
                        outs=[h2_gl[piece_base[j] : piece_base[j + 1], :].opt()],
                    )

            with tc.tile_pool(name="msg1", bufs=5) as mp1:
                agg_stage(h1_gl, HID, mp1, "m1", stage3_window, fire_ag2)

            # ---- stage 5 ----
            def stage5_window(w, msgs, moff):
                po = pp.tile([P, OUTP], f32, tag="acc128b")
                window_accum(w, msgs, moff, po, b2s, h2_loc, OUTP)
                os_ = wp.tile([P, OUT], f32, tag="os")
                nc.scalar.activation(
                    os_[:], po[:, :OUT], mybir.ActivationFunctionType.Copy,
                    scale=disS[:, w : w + 1],
                )
                nc.sync.dma_start(out=out_d[ts(w, P), :], in_=os_[:])

            with tc.tile_pool(name="msg2", bufs=5) as mp2:
                agg_stage(h2_gl, OUTP, mp2, "m2", stage5_window, None)

    nc.compile()
    return nc


def kernel(x, W1, b1, W2, b2, edge_index, _run_opts=None):
    from concourse.bass_utils import run_bass_kernel_spmd

    x = np.asarray(x)
    edge_index = np.asarray(edge_index)
    in_maps, meta = _prep(
        x, np.asarray(W1), np.asarray(b1), np.asarray(W2), np.asarray(b2), edge_index
    )
    nc = _build(meta)
    opts = dict(_run_opts or {})
    opts.pop("_bass_results", None)
    res = run_bass_kernel_spmd(nc, in_maps, core_ids=list(range(M)), **opts)
    NP, OUT = meta["NP"], meta["OUT"]
    out = np.concatenate(
        [res.results[c]["out"][:NP] for c in range(M)], axis=0
    ).astype(np.float32)
    if _run_opts is not None:
        _run_opts["_bass_results"] = res
    return out


# revision 19
# speedup vs baseline: 1.0894x; 1.0894x over previous
"""GCN (2-layer, PyG GCNConv semantics) on 8 Trainium2 NeuronCores.

Strategy (sharding_hint: shard nodes across cores, partition edges by dst):
  - Nodes sharded contiguously: core c owns dst rows [c*NP, (c+1)*NP).
  - Layer matmuls computed on the owning core (fp16 operands, fp32 PSUM).
  - Hidden tables (h1' = dis*x@W1, h2' = dis*relu(z1)@W2) are AllGathered
    in 4 pieces (overlapped with compute) so every core can gather
    messages for its own edges locally.
  - The symmetric norm dis[s]*dis[d] is factored: table rows are
    pre-scaled by dis[v]; the window PSUM is scaled by dis[d] on the way
    out (ACT scale); the bias is injected as (b/dis[d]) via a K=1 matmul
    that also initializes the accumulation group.
  - Aggregation out[dst] += h'[src_e] runs per 128-dst window: dma_gather
    pulls h'[src] rows for the window's non-self edges into SBUF (128
    edges per chunk), a 0/1 selection mask G[e, d] = (dst_rel[e]==d) is
    built with one DVE tensor_tensor per (window, half), and TensorE
    accumulates G.T @ msg into the window's PSUM tile.  Self-loops are
    the own-shard diagonal: one identity matmul on a contiguous DMA of
    the own h' tile.
  - All cores run one identical program: each (window, src-half) edge
    group is padded to CH_wh[w,h] chunks of 128 tokens, CH_wh being the
    max chunk count over the 8 cores (pad tokens have dst_rel=255 so the
    mask kills them).
"""

import math

import numpy as np

M = 8  # cores
P = 128  # partitions
AGP = 2  # all-gather pieces == gather table halves
BW = 4  # windows per gather batch


def _prep(x, W1, b1, W2, b2, edge_index):
    """Host-side sharding/layout (index manipulation + dtype casts only)."""
    N, IN = x.shape
    HID = W1.shape[1]
    OUT = W2.shape[1]
    OUTP = P
    assert N % M == 0
    NP = N // M
    NPAD = math.ceil(NP / P) * P
    NT = NPAD // P
    VROWS = M * NPAD

    src = np.asarray(edge_index[0], dtype=np.int64)
    dst = np.asarray(edge_index[1], dtype=np.int64)
    deg = (np.bincount(dst, minlength=N) + 1).astype(np.float32)
    dis = 1.0 / np.sqrt(deg)

    batches = [list(range(s, min(s + BW, NT))) for s in range(0, NT, BW)]
    nb = len(batches)
    pieces = []
    per = math.ceil(nb / AGP)
    for s in range(0, nb, per):
        pieces.append((s, min(s + per, nb)))
    piece_wins = [sum(len(batches[b]) for b in range(lo, hi)) for lo, hi in pieces]
    piece_rows = [wn * P for wn in piece_wins]
    piece_win_start = np.cumsum([0] + piece_wins)
    piece_base = np.cumsum([0] + [r * M for r in piece_rows])

    win_of_l = np.arange(NPAD) // P
    piece_of_win = np.zeros(NT, dtype=np.int64)
    for j in range(len(pieces)):
        piece_of_win[piece_win_start[j] : piece_win_start[j + 1]] = j

    sc, sl = src // NP, src % NP
    pj = piece_of_win[win_of_l[sl]]
    g = (
        piece_base[pj]
        + sc * np.array(piece_rows)[pj]
        + (sl - piece_win_start[pj] * P)
    )
    # src half == all-gather piece; local idx within the piece's global block
    half = pj
    lidx = g - np.array(piece_base)[pj]
    assert int(lidx.max(initial=0)) < 32768, "gather idx must fit int16"

    owner = dst // NP
    ldst = dst - owner * NP
    win = ldst // P

    gid = (owner * NT + win) * 2 + half
    order = np.lexsort((lidx, gid))
    lidx_s = lidx[order]
    drel_s = (ldst[order] % P).astype(np.float16)

    ngroups = M * NT * 2
    counts = np.bincount(gid[order], minlength=ngroups).reshape(M, NT, 2)
    # per-(window, half) chunk count: max over cores (identical SPMD program)
    CH_wh = np.ceil(counts.max(axis=0) / P).astype(np.int64)  # [NT, 2]
    CHMAX = max(1, int(CH_wh.max()))

    # token slots / chunk columns in kernel iteration order:
    # batch -> half -> window-in-batch -> chunk
    slot_base = np.zeros((NT, 2), dtype=np.int64)
    col_of = np.zeros((NT, 2), dtype=np.int64)
    tb = 0
    cb = 0
    for bwins in batches:
        for h in (0, 1):
            for w in bwins:
                slot_base[w, h] = tb
                col_of[w, h] = cb
                tb += int(CH_wh[w, h]) * P
                cb += int(CH_wh[w, h])
    TOK = int(tb)
    NCHUNK = int(cb)

    group_starts = np.zeros(ngroups + 1, dtype=np.int64)
    np.cumsum(counts.reshape(-1), out=group_starts[1:])

    in_maps = []
    f16 = np.float16
    w1f = np.ascontiguousarray(W1, dtype=f16)
    b1f = np.ascontiguousarray(
        np.broadcast_to(np.asarray(b1, dtype=f16).reshape(1, HID), (P, HID))
    )
    w2f = np.zeros((HID, OUTP), dtype=f16)
    w2f[:, :OUT] = W2.astype(f16)
    b2f = np.zeros((P, OUTP), dtype=f16)
    b2f[:, :OUT] = np.asarray(b2, dtype=f16).reshape(1, OUT)
    iota_np = np.ascontiguousarray(
        np.broadcast_to(
            np.tile(np.arange(P, dtype=f16), CHMAX).reshape(1, CHMAX * P),
            (P, CHMAX * P),
        )
    )

    for c in range(M):
        xt = np.zeros((IN, NPAD), dtype=f16)
        xt[:, :NP] = x[c * NP : (c + 1) * NP].T
        idx16 = np.zeros(TOK, dtype=np.int16)
        drel = np.full(TOK, 255.0, dtype=np.float16)
        for w in range(NT):
            for h in (0, 1):
                gi = (c * NT + w) * 2 + h
                s0, s1 = group_starts[gi], group_starts[gi + 1]
                k = s1 - s0
                base = slot_base[w, h]
                idx16[base : base + k] = lidx_s[s0:s1]
                drel[base : base + k] = drel_s[s0:s1]
        idx_w = np.tile(idx16.reshape(-1, 16).T, (8, 1))
        grel = np.ascontiguousarray(drel.reshape(-1, P).T)
        dloc = np.ones(NPAD, np.float32)
        dloc[:NP] = dis[c * NP : (c + 1) * NP]
        disS = np.ascontiguousarray(dloc.reshape(NT, P).T)
        invdC = np.ascontiguousarray((1.0 / dloc).astype(np.float32).reshape(NT, P).T)
        invd = (1.0 / dloc).astype(f16).reshape(1, NPAD)
        in_maps.append(
            {
                "xt": xt,
                "w1": w1f,
                "b1": b1f,
                "w2": w2f,
                "b2": b2f,
                "iota": np.array(iota_np),
                "idx": np.ascontiguousarray(idx_w),
                "grel": grel,
                "disS": disS,
                "invdC": invdC,
                "invd": invd,
            }
        )

    meta = dict(
        N=N, IN=IN, HID=HID, OUT=OUT, OUTP=OUTP, NP=NP, NPAD=NPAD, NT=NT,
        VROWS=VROWS, CHMAX=CHMAX, TOK=TOK, NCHUNK=NCHUNK,
        CH_wh=[[int(CH_wh[w, 0]), int(CH_wh[w, 1])] for w in range(NT)],
        col_of=[[int(col_of[w, 0]), int(col_of[w, 1])] for w in range(NT)],
        batches=batches, piece_rows=piece_rows,
        piece_win_start=[int(v) for v in piece_win_start],
        piece_base=[int(v) for v in piece_base],
    )
    return in_maps, meta


def _build(meta):
    import os

    import concourse.mybir as mybir
    import concourse.tile as tile
    from concourse import bacc
    from concourse.bass import ts
    from concourse.masks import make_identity

    IN, HID, OUT, OUTP = meta["IN"], meta["HID"], meta["OUT"], meta["OUTP"]
    NPAD, NT, VROWS = meta["NPAD"], meta["NT"], meta["VROWS"]
    CHMAX, TOK, NCHUNK = meta["CHMAX"], meta["TOK"], meta["NCHUNK"]
    CH_wh = meta["CH_wh"]
    col_of = meta["col_of"]
    batches = meta["batches"]
    piece_rows = meta["piece_rows"]
    piece_win_start = meta["piece_win_start"]
    piece_base = meta["piece_base"]
    NPIECE = len(piece_rows)
    KT = IN // P
    HC = HID // P
    f16 = mybir.dt.float16
    f32 = mybir.dt.float32

    NQ = 4
    GN = 1024
    SP = os.environ.get("GCN_SP", "1") == "1"
    nc = bacc.Bacc(
        "TRN2",
        target_bir_lowering=False,
        debug=False,
        num_devices=M,
        num_swdge_queues=NQ,
    )

    xt_d = nc.dram_tensor("xt", [IN, NPAD], f16, kind="ExternalInput")
    w1_d = nc.dram_tensor("w1", [IN, HID], f16, kind="ExternalInput")
    b1_d = nc.dram_tensor("b1", [P, HID], f16, kind="ExternalInput")
    w2_d = nc.dram_tensor("w2", [HID, OUTP], f16, kind="ExternalInput")
    b2_d = nc.dram_tensor("b2", [P, OUTP], f16, kind="ExternalInput")
    iota_d = nc.dram_tensor("iota", [P, CHMAX * P], f16, kind="ExternalInput")
    idx_d = nc.dram_tensor("idx", [P, TOK // 16], mybir.dt.int16, kind="ExternalInput")
    grel_d = nc.dram_tensor("grel", [P, NCHUNK], f16, kind="ExternalInput")
    disS_d = nc.dram_tensor("disS", [P, NT], f32, kind="ExternalInput")
    invd_d = nc.dram_tensor("invd", [1, NPAD], f16, kind="ExternalInput")
    invdC_d = nc.dram_tensor("invdC", [P, NT], f32, kind="ExternalInput")
    out_d = nc.dram_tensor("out", [NPAD, OUT], f32, kind="ExternalOutput")

    h1_loc = [
        nc.dram_tensor(f"h1_loc{j}", [piece_rows[j], HID], f16)
        for j in range(NPIECE)
    ]
    h2_loc = [
        nc.dram_tensor(f"h2_loc{j}", [piece_rows[j], OUTP], f16)
        for j in range(NPIECE)
    ]
    assert NPIECE <= 2
    h1_gl = [
        nc.dram_tensor(f"h1_gl{j}", [piece_rows[j] * M, HID], f16,
                       addr_space="Shared")
        for j in range(NPIECE)
    ]
    h2_gl = [
        nc.dram_tensor(f"h2_gl{j}", [piece_rows[j] * M, OUTP], f16,
                       addr_space="Shared")
        for j in range(NPIECE)
    ]

    rg = [list(range(M))]

    def win_piece(w):
        for j in range(NPIECE):
            if piece_win_start[j] <= w < piece_win_start[j + 1]:
                return j, w - piece_win_start[j]
        raise AssertionError(w)

    with tile.TileContext(nc) as tc:
        with (
            tc.tile_pool(name="const", bufs=1) as cp,
            tc.tile_pool(name="work", bufs=3) as wp,
            tc.tile_pool(name="gpool", bufs=4) as gp,
            tc.tile_pool(name="idxp", bufs=2) as idxp,
            tc.tile_pool(name="psum", bufs=2, space="PSUM") as pp,
        ):
            # ---- constants ----
            w1t = cp.tile([P, KT, HID], f16)
            nc.sync.dma_start(
                out=w1t[:], in_=w1_d[:, :].rearrange("(k p) h -> p k h", p=P)
            )
            w2t = cp.tile([P, HC, OUTP], f16)
            nc.sync.dma_start(
                out=w2t[:], in_=w2_d[:, :].rearrange("(k p) o -> p k o", p=P)
            )
            iota_t = cp.tile([P, CHMAX * P], f16)
            nc.sync.dma_start(out=iota_t[:], in_=iota_d[:, :])
            ident = cp.tile([P, P], f16)
            make_identity(nc, ident[:])
            b1s = cp.tile([P, HID], f16)
            nc.sync.dma_start(out=b1s[:], in_=b1_d[:, :])
            b2s = cp.tile([P, OUTP], f16)
            nc.sync.dma_start(out=b2s[:], in_=b2_d[:, :])
            grelS = cp.tile([P, NCHUNK], f16)
            nc.sync.dma_start(out=grelS[:], in_=grel_d[:, :])
            disS = cp.tile([P, NT], f32)
            nc.sync.dma_start(out=disS[:], in_=disS_d[:, :])
            invd = cp.tile([1, NPAD], f16)
            nc.sync.dma_start(out=invd[:], in_=invd_d[:, :])
            invdC = cp.tile([P, NT], f32)
            nc.sync.dma_start(out=invdC[:], in_=invdC_d[:, :])

            # ---- stage 1 ----
            for nt in range(NT):
                j, wo = win_piece(nt)
                xtt = wp.tile([P, KT, P], f16, tag="xtt")
                nc.sync.dma_start(
                    out=xtt[:],
                    in_=xt_d[:, ts(nt, P)].rearrange("(k p) n -> p k n", p=P),
                )
                ph = pp.tile([P, HID], f32, tag="acc256")
                for k in range(KT):
                    nc.tensor.matmul(
                        ph[:],
                        lhsT=xtt[:, k, :],
                        rhs=w1t[:, k, :],
                        start=(k == 0),
                        stop=(k == KT - 1),
                    )
                h1s = wp.tile([P, HID], f16, tag="h1s")
                nc.scalar.activation(
                    h1s[:], ph[:], mybir.ActivationFunctionType.Copy,
                    scale=disS[:, nt : nt + 1],
                )
                nc.sync.dma_start(out=h1_loc[j][ts(wo, P), :], in_=h1s[:])

            # ---- stage 2: AllGather h1, one piece per gather half ----
            for j in range(NPIECE):
                nc.gpsimd.collective_compute(
                    "AllGather",
                    mybir.AluOpType.bypass,
                    replica_groups=rg,
                    ins=[h1_loc[j].ap().opt()],
                    outs=[h1_gl[j].ap().opt()],
                )

            qctr = [0]

            def build_GW(w, h):
                chw = CH_wh[w][h]
                c0 = col_of[w][h]
                GW = gp.tile([P, chw * P], f16, tag="GW")
                nc.vector.tensor_tensor(
                    out=GW[:].rearrange("p (c e) -> p c e", e=P),
                    in0=iota_t[:, : chw * P].rearrange("p (c e) -> p c e", e=P),
                    in1=grelS[:, c0 : c0 + chw].to_broadcast([P, chw, P]),
                    op=mybir.AluOpType.is_equal,
                )
                return GW

            def window_accum(w, msgs, moff, acc, bvec, own_loc, own_elem):
                """(self + bias/dis) identity matmul + edge-chunk matmuls."""
                j, wo = win_piece(w)
                own = wp.tile([P, own_elem], f16, tag=f"own{own_elem}")
                nc.sync.dma_start(out=own[:], in_=own_loc[j][ts(wo, P), :])
                ownb = wp.tile([P, own_elem], f16, tag=f"ownb{own_elem}")
                # ownb = bvec * (1/dis[d]) + own   (bias pre-divided by dis)
                nc.vector.scalar_tensor_tensor(
                    out=ownb[:],
                    in0=bvec[:],
                    scalar=invdC[:, w : w + 1],
                    in1=own[:],
                    op0=mybir.AluOpType.mult,
                    op1=mybir.AluOpType.add,
                )
                mms = []
                for h in (0, 1):
                    if CH_wh[w][h] == 0:
                        continue
                    GW = build_GW(w, h)
                    for c in range(CH_wh[w][h]):
                        mms.append((GW, h, c))
                nc.tensor.matmul(
                    acc[:], lhsT=ident[:], rhs=ownb[:],
                    start=True, stop=(len(mms) == 0),
                )
                for k, (GW, h, c) in enumerate(mms):
                    nc.tensor.matmul(
                        acc[:],
                        lhsT=GW[:, ts(c, P)],
                        rhs=msgs[h][:, moff[h] + c, :],
                        start=False,
                        stop=(k == len(mms) - 1),
                    )

            def agg_stage(table, elem, msg_pool, msg_tag, consume, post_window):
                tok_base = 0
                for bwins in batches:
                    chA = sum(CH_wh[w][0] for w in bwins)
                    chB = sum(CH_wh[w][1] for w in bwins)
                    btokA, btokB = chA * P, chB * P
                    btot = btokA + btokB
                    idx_t = idxp.tile([P, btot // 16], mybir.dt.int16, tag="idx")
                    nc.sync.dma_start(
                        out=idx_t[:],
                        in_=idx_d[:, tok_base // 16 : (tok_base + btot) // 16],
                    )
                    msgs = []
                    for h, btok in ((0, btokA), (1, btokB)):
                        nch = btok // P
                        if nch == 0 or h >= NPIECE:
                            msgs.append(None)
                            continue
                        mt = msg_pool.tile([P, nch, elem], f16, tag=msg_tag)
                        i00 = 0 if h == 0 else btokA
                        for off in range(0, btok, GN):
                            gn = min(GN, btok - off)
                            i0 = i00 + off
                            nc.gpsimd.dma_gather(
                                out_ap=mt[:, off // P : (off + gn) // P, :],
                                in_ap=table[h][:, :],
                                idxs_ap=idx_t[:, i0 // 16 : (i0 + gn) // 16],
                                num_idxs=gn,
                                num_idxs_reg=gn,
                                elem_size=elem,
                                queue_num=qctr[0] % NQ,
                                single_packet=SP,
                            )
                            qctr[0] += 1
                        msgs.append(mt)
                    coff = [0, 0]
                    for w in bwins:
                        consume(w, msgs, (coff[0], coff[1]))
                        coff[0] += CH_wh[w][0]
                        coff[1] += CH_wh[w][1]
                        if post_window is not None:
                            post_window(w)
                    tok_base += btot

            # ---- stage 3 + chunked AllGather h2 ----
            def stage3_window(w, msgs, moff):
                j, wo = win_piece(w)
                pz = pp.tile([P, HID], f32, tag="acc256")
                window_accum(w, msgs, moff, pz, b1s, h1_loc, HID)
                z1r = wp.tile([P, HID], f16, tag="z1r")
                nc.scalar.activation(
                    z1r[:], pz[:], mybir.ActivationFunctionType.Relu,
                    scale=disS[:, w : w + 1],
                )
                ph2 = pp.tile([P, OUTP], f32, tag="acc128b")
                for k in range(HC):
                    pt = pp.tile([P, P], f16, tag="acc128t")
                    nc.tensor.transpose(pt[:], z1r[:, ts(k, P)], ident[:])
                    zt = wp.tile([P, P], f16, tag="zt")
                    nc.vector.tensor_copy(zt[:], pt[:])
                    nc.tensor.matmul(
                        ph2[:],
                        lhsT=zt[:],
                        rhs=w2t[:, k, :],
                        start=(k == 0),
                        stop=(k == HC - 1),
                    )
                h2s = wp.tile([P, OUTP], f16, tag="h2s")
                nc.scalar.activation(
                    h2s[:], ph2[:], mybir.ActivationFunctionType.Copy,
                    scale=disS[:, w : w + 1],
                )
                nc.sync.dma_start(out=h2_loc[j][ts(wo, P), :], in_=h2s[:])

            done_pieces = set()

            def fire_ag2(w):
                j, wo = win_piece(w)
                if wo == piece_rows[j] // P - 1 and j not in done_pieces:
                    done_pieces.add(j)
                    nc.gpsimd.collective_compute(
                        "AllGather",
                        mybir.AluOpType.bypass,
                        replica_groups=rg,
                        ins=[h2_loc[j].ap().opt()],
                        outs=[h2_gl[j].ap().opt()],
                    )

            with tc.tile_pool(name="msg1", bufs=5) as mp1:
                agg_stage(h1_gl, HID, mp1, "m1", stage3_window, fire_ag2)

            # ---- stage 5 ----
            def stage5_window(w, msgs, moff):
                po = pp.tile([P, OUTP], f32, tag="acc128b")
                window_accum(w, msgs, moff, po, b2s, h2_loc, OUTP)
                os_ = wp.tile([P, OUT], f32, tag="os")
                nc.scalar.activation(
                    os_[:], po[:, :OUT], mybir.ActivationFunctionType.Copy,
                    scale=disS[:, w : w + 1],
                )
                nc.sync.dma_start(out=out_d[ts(w, P), :], in_=os_[:])

            with tc.tile_pool(name="msg2", bufs=5) as mp2:
                agg_stage(h2_gl, OUTP, mp2, "m2", stage5_window, None)

    nc.compile()
    return nc


def kernel(x, W1, b1, W2, b2, edge_index, _run_opts=None):
    from concourse.bass_utils import run_bass_kernel_spmd

    x = np.asarray(x)
    edge_index = np.asarray(edge_index)
    in_maps, meta = _prep(
        x, np.asarray(W1), np.asarray(b1), np.asarray(W2), np.asarray(b2), edge_index
    )
    nc = _build(meta)
    opts = dict(_run_opts or {})
    opts.pop("_bass_results", None)
    res = run_bass_kernel_spmd(nc, in_maps, core_ids=list(range(M)), **opts)
    NP, OUT = meta["NP"], meta["OUT"]
    out = np.concatenate(
        [res.results[c]["out"][:NP] for c in range(M)], axis=0
    ).astype(np.float32)
    if _run_opts is not None:
        _run_opts["_bass_results"] = res
    return out
